# revision 4
# baseline (speedup 1.0000x reference)
"""ColdDiffusion q_sample with anchor matching (retrieval kNN) on 8 trn2 cores.

For each of B*N=32768 queries (8-dim), find the nearest of M=8192 anchors
(argmin Euclidean), gather it, and return sa[t]*x + sb[t]*matched.

Strategy: data-parallel over batch (2 batches = 4096 queries per core),
anchors replicated. argmin dist == argmax score where
score = x.a - ||a||^2/2, computed as a K=9 augmented matmul on the PE
([x,1] . [a, -||a||^2/2]). Per 128-query tile: PE writes [128, 8192]
scores to PSUM in 2 halves, ACT copies each half to SBUF, DVE computes
the row max via tensor_scalar max-accum (2x mode) and recovers the exact
first-occurrence argmax with one max_index pass (1x). indirect DMA
gathers the matched anchor rows; the final AXPY is batched across tiles.
"""

import numpy as np

P = 128
D = 8          # feature dim (4*2 flattened)
K = 9          # augmented contraction dim
B_FULL, N_FULL = 16, 2048
M_FULL = 8192
N_CORES = 8
Q_CORE = B_FULL * N_FULL // N_CORES   # 4096 queries per core

_CACHE = {}


def build_nc(n_qtiles, m, chunk=512, half_chunks=4, repeat=1):
    """Build the per-core Bass module. All cores run this same program (SPMD).

    repeat>1 wraps the compute in a For_i loop (timing harness only).
    """
    import concourse.bacc as bacc
    import concourse.bass as bass
    import concourse.mybir as mybir
    import concourse.tile as tile
    from contextlib import nullcontext

    f32 = mybir.dt.float32
    alu = mybir.AluOpType
    half = chunk * half_chunks            # 4096
    n_half = m // half                    # halves per row (2)
    q = n_qtiles * P

    nc = bacc.Bacc(None, target_bir_lowering=False)

    xaT = nc.dram_tensor("xaT", [K, q], f32, kind="ExternalInput")
    aaT = nc.dram_tensor("aaT", [K, m], f32, kind="ExternalInput")
    x_nat = nc.dram_tensor("x_nat", [P, n_qtiles * D], f32, kind="ExternalInput")
    sa8 = nc.dram_tensor("sa8", [P, n_qtiles * D], f32, kind="ExternalInput")
    sb8 = nc.dram_tensor("sb8", [P, n_qtiles * D], f32, kind="ExternalInput")
    anch = nc.dram_tensor("anch", [m, D], f32, kind="ExternalInput")
    out_d = nc.dram_tensor("out", [P, n_qtiles * D], f32, kind="ExternalOutput")

    with tile.TileContext(nc) as tc:
        with (
            tc.tile_pool(name="const", bufs=1) as cpool,
            tc.tile_pool(name="scores", bufs=2) as spool,
            tc.tile_pool(name="scratch", bufs=2) as scpool,
            tc.tile_pool(name="small", bufs=3) as smpool,
            tc.tile_pool(name="psum", bufs=2, space="PSUM") as ppool,
        ):
            # ---- load constants ----
            xaT_s = cpool.tile([K, q], f32)
            aaT_s = cpool.tile([K, m], f32)
            x_nat_s = cpool.tile([P, n_qtiles * D], f32)
            sa_s = cpool.tile([P, n_qtiles * D], f32)
            sb_s = cpool.tile([P, n_qtiles * D], f32)
            nc.sync.dma_start(out=xaT_s[:], in_=xaT[:])
            nc.sync.dma_start(out=aaT_s[:], in_=aaT[:])
            nc.sync.dma_start(out=x_nat_s[:], in_=x_nat[:])
            nc.sync.dma_start(out=sa_s[:], in_=sa8[:])
            nc.sync.dma_start(out=sb_s[:], in_=sb8[:])

            idx_all = cpool.tile([P, n_qtiles], mybir.dt.int32)
            matched = cpool.tile([P, n_qtiles * D], f32)
            t1 = cpool.tile([P, n_qtiles * D], f32)
            out_all = cpool.tile([P, n_qtiles * D], f32)

            loop = tc.For_i(0, repeat, 1) if repeat > 1 else nullcontext()
            with loop:
                for t in range(n_qtiles):
                    scores = spool.tile([P, m], f32, tag="scores")
                    qbuf = smpool.tile([P, 8], f32, tag="qbuf")
                    nc.vector.memset(qbuf[:], -3.0e38)

                    lhsT = xaT_s[:, t * P:(t + 1) * P]
                    for j in range(n_half):
                        ps = ppool.tile([P, half], f32, tag="ps")
                        for c in range(half_chunks):
                            m0 = j * half + c * chunk
                            nc.tensor.matmul(
                                out=ps[:, c * chunk:(c + 1) * chunk],
                                lhsT=lhsT,
                                rhs=aaT_s[:, m0:m0 + chunk],
                                start=True, stop=True,
                            )
                        # PSUM -> SBUF copy on ACT
                        nc.scalar.copy(
                            out=scores[:, j * half:(j + 1) * half], in_=ps[:]
                        )
                        # per-half running max (DVE tensor_scalar accum, 2x)
                        sc = scpool.tile([P, half], f32, tag="sc")
                        nc.vector.tensor_scalar(
                            sc[:], scores[:, j * half:(j + 1) * half],
                            1.0, None, op0=alu.mult, op1=alu.max,
                            accum_out=qbuf[:, j:j + 1],
                        )

                    # top-8 needles (slot 0 = row max), first-occurrence index
                    v8 = smpool.tile([P, 8], f32, tag="v8")
                    nc.vector.max(out=v8[:], in_=qbuf[:])
                    jidx = smpool.tile([P, 8], mybir.dt.uint32, tag="jidx")
                    nc.vector.max_index(out=jidx[:], in_max=v8[:],
                                        in_values=scores[:])
                    nc.vector.tensor_copy(out=idx_all[:, t:t + 1],
                                          in_=jidx[:, 0:1])

                    # gather matched anchor rows from DRAM
                    nc.gpsimd.indirect_dma_start(
                        out=matched[:, t * D:(t + 1) * D],
                        out_offset=None,
                        in_=anch[:],
                        in_offset=bass.IndirectOffsetOnAxis(
                            ap=idx_all[:, t:t + 1], axis=0
                        ),
                    )

                # batched AXPY: out = sa*x + sb*matched
                nc.vector.tensor_tensor(out=t1[:], in0=x_nat_s[:], in1=sa_s[:],
                                        op=alu.mult)
                nc.vector.tensor_tensor(out=out_all[:], in0=matched[:],
                                        in1=sb_s[:], op=alu.mult)
                nc.vector.tensor_tensor(out=out_all[:], in0=out_all[:],
                                        in1=t1[:], op=alu.add)
                nc.sync.dma_start(out=out_d[:], in_=out_all[:])

    nc.compile()
    return nc


def _prep_core_inputs(xf, af, sa_q, sb_q, n_qtiles, m):
    """Host-side input prep for one core's query slice."""
    q = n_qtiles * P
    assert xf.shape == (q, D)
    xaT = np.concatenate([xf.T, np.ones((1, q), np.float32)]).astype(np.float32)
    aaT = np.concatenate(
        [af.T, (-0.5 * (af * af).sum(1))[None, :]]
    ).astype(np.float32)
    x_nat = xf.reshape(n_qtiles, P, D).transpose(1, 0, 2).reshape(P, n_qtiles * D)
    sa8 = np.repeat(sa_q.reshape(n_qtiles, P).T, D, axis=1)   # [P, n_qtiles*D]
    sb8 = np.repeat(sb_q.reshape(n_qtiles, P).T, D, axis=1)
    return {
        "xaT": np.ascontiguousarray(xaT),
        "aaT": np.ascontiguousarray(aaT),
        "x_nat": np.ascontiguousarray(x_nat),
        "sa8": np.ascontiguousarray(sa8),
        "sb8": np.ascontiguousarray(sb8),
        "anch": np.ascontiguousarray(af),
    }


def run_cores(nc, in_maps, trace=False):
    from concourse.bass_utils import run_bass_kernel_spmd
    return run_bass_kernel_spmd(
        nc, in_maps, core_ids=list(range(len(in_maps))), trace=trace
    )


def _unpack_core_out(r, n_qtiles):
    return r.reshape(P, n_qtiles, D).transpose(1, 0, 2).reshape(n_qtiles * P, D)


def kernel(x_start, anchors, sqrt_alphas_cumprod, sqrt_one_minus_alphas_cumprod, t):
    x_start = np.asarray(x_start, dtype=np.float32)
    anchors = np.asarray(anchors, dtype=np.float32)
    sa = np.asarray(sqrt_alphas_cumprod, dtype=np.float32)
    sb = np.asarray(sqrt_one_minus_alphas_cumprod, dtype=np.float32)
    t = np.asarray(t)

    B, N = x_start.shape[:2]
    xf = x_start.reshape(B * N, D)
    af = anchors.reshape(anchors.shape[0], D)
    m = af.shape[0]
    sa_q = np.repeat(sa[t], N).astype(np.float32)   # [B*N]
    sb_q = np.repeat(sb[t], N).astype(np.float32)

    n_qtiles = Q_CORE // P
    key = (n_qtiles, m)
    if key not in _CACHE:
        _CACHE[key] = build_nc(n_qtiles, m)
    nc = _CACHE[key]

    in_maps = []
    for c in range(N_CORES):
        sl = slice(c * Q_CORE, (c + 1) * Q_CORE)
        in_maps.append(_prep_core_inputs(xf[sl], af, sa_q[sl], sb_q[sl],
                                         n_qtiles, m))

    results = run_cores(nc, in_maps).results

    out = np.empty((B * N, D), np.float32)
    for c in range(N_CORES):
        out[c * Q_CORE:(c + 1) * Q_CORE] = _unpack_core_out(
            results[c]["out"], n_qtiles
        )
    return out.reshape(B, N, 4, 2)


# revision 8
# speedup vs baseline: 1.4943x; 1.4943x over previous
"""ColdDiffusion q_sample with anchor matching (retrieval kNN) on 8 trn2 cores.

For each of B*N=32768 queries (8-dim), find the nearest of M=8192 anchors
(argmin Euclidean), gather it, and return sa[t]*x + sb[t]*matched.

Strategy: data-parallel over batch (2 batches = 4096 queries per core),
anchors replicated. argmin dist == argmax score where
score = x.a - ||a||^2/2, computed as a K=9 augmented matmul on the PE
([x,1] . [a, -||a||^2/2]). Per 128-query tile: PE writes [128, 8192]
scores to PSUM in 2 halves, ACT copies each half to SBUF, DVE computes
the row max via tensor_scalar max-accum (2x mode) and recovers the exact
first-occurrence argmax with one max_index pass (1x). indirect DMA
gathers the matched anchor rows; the final AXPY is batched across tiles.
"""

import numpy as np

P = 128
D = 8          # feature dim (4*2 flattened)
K = 51         # bf16-split contraction: 6 limb cross-products (8 rows each) + 3 bias rows
B_FULL, N_FULL = 16, 2048
M_FULL = 8192
N_CORES = 8
Q_CORE = B_FULL * N_FULL // N_CORES   # 4096 queries per core

_CACHE = {}


def build_nc(n_qtiles, m, chunk=512, half_chunks=4, repeat=1):
    """Build the per-core Bass module. All cores run this same program (SPMD).

    repeat>1 wraps the compute in a For_i loop (timing harness only).
    """
    import concourse.bacc as bacc
    import concourse.bass as bass
    import concourse.mybir as mybir
    import concourse.tile as tile
    from contextlib import nullcontext

    f32 = mybir.dt.float32
    bf16 = mybir.dt.bfloat16
    alu = mybir.AluOpType
    half = chunk * half_chunks            # 4096
    n_half = m // half                    # halves per row (2)
    q = n_qtiles * P

    nc = bacc.Bacc(None, target_bir_lowering=False)

    xaT = nc.dram_tensor("xaT", [K, q], bf16, kind="ExternalInput")
    aaT = nc.dram_tensor("aaT", [K, m], bf16, kind="ExternalInput")
    x_nat = nc.dram_tensor("x_nat", [P, n_qtiles * D], f32, kind="ExternalInput")
    sa8 = nc.dram_tensor("sa8", [P, n_qtiles * D], f32, kind="ExternalInput")
    sb8 = nc.dram_tensor("sb8", [P, n_qtiles * D], f32, kind="ExternalInput")
    anch = nc.dram_tensor("anch", [m, D], f32, kind="ExternalInput")
    out_d = nc.dram_tensor("out", [P, n_qtiles * D], f32, kind="ExternalOutput")

    with tile.TileContext(nc) as tc:
        with (
            tc.tile_pool(name="const", bufs=1) as cpool,
            tc.tile_pool(name="scores", bufs=2) as spool,
            tc.tile_pool(name="scratch", bufs=2) as scpool,
            tc.tile_pool(name="small", bufs=3) as smpool,
            tc.tile_pool(name="psum", bufs=2, space="PSUM") as ppool,
        ):
            # ---- load constants ----
            xaT_s = cpool.tile([K, q], bf16)
            aaT_s = cpool.tile([K, m], bf16)
            x_nat_s = cpool.tile([P, n_qtiles * D], f32)
            sa_s = cpool.tile([P, n_qtiles * D], f32)
            sb_s = cpool.tile([P, n_qtiles * D], f32)
            nc.sync.dma_start(out=xaT_s[:], in_=xaT[:])
            nc.sync.dma_start(out=aaT_s[:], in_=aaT[:])
            nc.sync.dma_start(out=x_nat_s[:], in_=x_nat[:])
            nc.sync.dma_start(out=sa_s[:], in_=sa8[:])
            nc.sync.dma_start(out=sb_s[:], in_=sb8[:])

            idx_all = cpool.tile([P, n_qtiles], mybir.dt.int32)
            matched = cpool.tile([P, n_qtiles * D], f32)
            t1 = cpool.tile([P, n_qtiles * D], f32)
            out_all = cpool.tile([P, n_qtiles * D], f32)

            loop = tc.For_i(0, repeat, 1) if repeat > 1 else nullcontext()
            with loop:
                for t in range(n_qtiles):
                    scores = spool.tile([P, m], f32, tag="scores")
                    qbuf = smpool.tile([P, 8], f32, tag="qbuf")
                    nc.vector.memset(qbuf[:], -3.0e38)

                    lhsT = xaT_s[:, t * P:(t + 1) * P]
                    for j in range(n_half):
                        ps = ppool.tile([P, half], f32, tag="ps")
                        for c in range(half_chunks):
                            m0 = j * half + c * chunk
                            nc.tensor.matmul(
                                out=ps[:, c * chunk:(c + 1) * chunk],
                                lhsT=lhsT,
                                rhs=aaT_s[:, m0:m0 + chunk],
                                start=True, stop=True,
                            )
                        # PSUM -> SBUF copy on ACT
                        nc.scalar.copy(
                            out=scores[:, j * half:(j + 1) * half], in_=ps[:]
                        )
                        # per-half running max (DVE tensor_scalar accum, 2x)
                        sc = scpool.tile([P, half], f32, tag="sc")
                        nc.vector.tensor_scalar(
                            sc[:], scores[:, j * half:(j + 1) * half],
                            1.0, None, op0=alu.mult, op1=alu.max,
                            accum_out=qbuf[:, j:j + 1],
                        )

                    # top-8 needles (slot 0 = row max), first-occurrence index
                    v8 = smpool.tile([P, 8], f32, tag="v8")
                    nc.vector.max(out=v8[:], in_=qbuf[:])
                    jidx = smpool.tile([P, 8], mybir.dt.uint32, tag="jidx")
                    nc.vector.max_index(out=jidx[:], in_max=v8[:],
                                        in_values=scores[:])
                    nc.vector.tensor_copy(out=idx_all[:, t:t + 1],
                                          in_=jidx[:, 0:1])

                    # gather matched anchor rows from DRAM
                    nc.gpsimd.indirect_dma_start(
                        out=matched[:, t * D:(t + 1) * D],
                        out_offset=None,
                        in_=anch[:],
                        in_offset=bass.IndirectOffsetOnAxis(
                            ap=idx_all[:, t:t + 1], axis=0
                        ),
                    )

                # batched AXPY: out = sa*x + sb*matched
                nc.vector.tensor_tensor(out=t1[:], in0=x_nat_s[:], in1=sa_s[:],
                                        op=alu.mult)
                nc.vector.tensor_tensor(out=out_all[:], in0=matched[:],
                                        in1=sb_s[:], op=alu.mult)
                nc.vector.tensor_tensor(out=out_all[:], in0=out_all[:],
                                        in1=t1[:], op=alu.add)
                nc.sync.dma_start(out=out_d[:], in_=out_all[:])

    nc.compile()
    return nc


def _bf16_limbs(x):
    """Split f32 array into 3 bf16 limbs with x ~= l0+l1+l2 (covers the
    full f32 mantissa; each limb product is exact in f32)."""
    import ml_dtypes
    bf = ml_dtypes.bfloat16
    l0 = x.astype(bf)
    r = x - l0.astype(np.float32)
    l1 = r.astype(bf)
    r = r - l1.astype(np.float32)
    l2 = r.astype(bf)
    return l0, l1, l2


def _prep_core_inputs(xf, af, sa_q, sb_q, n_qtiles, m):
    """Host-side input prep for one core's query slice.

    Builds the K=51 bf16-split operands: score = x.a - ||a||^2/2 with
    x = x0+x1+x2, a = a0+a1+a2 (bf16 limbs), keeping cross products with
    i+j<=2 plus the 3 bias limbs (paired against constant-1 rows).
    """
    import ml_dtypes
    bf = ml_dtypes.bfloat16
    q = n_qtiles * P
    assert xf.shape == (q, D)
    x0, x1, x2 = _bf16_limbs(xf.T)             # each [D, q]
    a0, a1, a2 = _bf16_limbs(af.T)             # each [D, m]
    b = (-0.5 * (af.astype(np.float64) * af).sum(1)).astype(np.float32)
    b0, b1, b2 = _bf16_limbs(b[None, :])       # each [1, m]
    ones = np.ones((1, q), dtype=bf)
    # row pairing: (x-side, a-side) rows multiply and accumulate in PSUM
    xaT = np.concatenate(
        [x0, ones, x0, ones, x0, ones, x1, x1, x2]
    ).astype(bf)                               # [51, q]
    aaT = np.concatenate(
        [a0, b0, a1, b1, a2, b2, a0, a1, a0]
    ).astype(bf)                               # [51, m]
    x_nat = xf.reshape(n_qtiles, P, D).transpose(1, 0, 2).reshape(P, n_qtiles * D)
    sa8 = np.repeat(sa_q.reshape(n_qtiles, P).T, D, axis=1)   # [P, n_qtiles*D]
    sb8 = np.repeat(sb_q.reshape(n_qtiles, P).T, D, axis=1)
    return {
        "xaT": np.ascontiguousarray(xaT),
        "aaT": np.ascontiguousarray(aaT),
        "x_nat": np.ascontiguousarray(x_nat),
        "sa8": np.ascontiguousarray(sa8),
        "sb8": np.ascontiguousarray(sb8),
        "anch": np.ascontiguousarray(af),
    }


def run_cores(nc, in_maps, trace=False):
    from concourse.bass_utils import run_bass_kernel_spmd
    return run_bass_kernel_spmd(
        nc, in_maps, core_ids=list(range(len(in_maps))), trace=trace
    )


def _unpack_core_out(r, n_qtiles):
    return r.reshape(P, n_qtiles, D).transpose(1, 0, 2).reshape(n_qtiles * P, D)


def kernel(x_start, anchors, sqrt_alphas_cumprod, sqrt_one_minus_alphas_cumprod, t):
    x_start = np.asarray(x_start, dtype=np.float32)
    anchors = np.asarray(anchors, dtype=np.float32)
    sa = np.asarray(sqrt_alphas_cumprod, dtype=np.float32)
    sb = np.asarray(sqrt_one_minus_alphas_cumprod, dtype=np.float32)
    t = np.asarray(t)

    B, N = x_start.shape[:2]
    xf = x_start.reshape(B * N, D)
    af = anchors.reshape(anchors.shape[0], D)
    m = af.shape[0]
    sa_q = np.repeat(sa[t], N).astype(np.float32)   # [B*N]
    sb_q = np.repeat(sb[t], N).astype(np.float32)

    n_qtiles = Q_CORE // P
    key = (n_qtiles, m)
    if key not in _CACHE:
        _CACHE[key] = build_nc(n_qtiles, m)
    nc = _CACHE[key]

    in_maps = []
    for c in range(N_CORES):
        sl = slice(c * Q_CORE, (c + 1) * Q_CORE)
        in_maps.append(_prep_core_inputs(xf[sl], af, sa_q[sl], sb_q[sl],
                                         n_qtiles, m))

    results = run_cores(nc, in_maps).results

    out = np.empty((B * N, D), np.float32)
    for c in range(N_CORES):
        out[c * Q_CORE:(c + 1) * Q_CORE] = _unpack_core_out(
            results[c]["out"], n_qtiles
        )
    return out.reshape(B, N, 4, 2)
